# revision 27
# baseline (speedup 1.0000x reference)
"""Multi-head attention (B=2, S=2048, D=1024, H=16, Dk=64) on 8 TRN2 cores.

Sharding: batch-split x head-TP.  Core c handles batch c//4 and heads
hs*4..hs*4+3 where hs = c%4 (256 projection dims = 2 "ob" blocks of 128).

The PE clock-gate (HAM) only unthrottles for full-array matmuls, so every
attention matmul is padded to 128x128:
  - scores: per-head K tiles kpA/kpB hold the head's 64 k-dims zero-padded
    to 128 partitions (zeros annihilate the other head's q rows), so
    scoresT = kpad.T @ qT runs K=128 full-array.
  - PV: v_aug columns padded with 1.0 to M=128; PSUM rows 0:64 = o,
    rows 64:128 all = softmax row-sum (the 1-columns), which feeds
    reciprocal_approx_fast directly -- no broadcast matmul needed.
Each core:
  1. projects k/q/v = (W_slice.T @ x.T) for its 4 heads
  2. transposes vT into per-(ob,h) [j, d] blocks (cols 64:128 = 1.0)
  3. pipelined attention per (ob, half): scoresT -> exp (FD=1024 ACT)
     -> PV accumulate [128, 1024] PSUM -> 1/rowsum -> normalize into oT2
  4. partialT = Wo_slice.T @ oT  (K=256 accumulated over both obs)
Host sums 4 partials per batch, adds bo, transposes back.
All matmuls fp16 operands with fp32 PSUM accumulation.
"""

import numpy as np

D = 1024
S = 2048  # tokens per batch (= per core)
B = 2
N_CORES = 8

_CACHE = {}


def _build_nc(mm_dtype="float16"):
    import concourse.bacc as bacc
    import concourse.mybir as mybir
    import concourse.tile as tile

    dt = mybir.dt
    f32 = dt.float32
    mmdt = getattr(dt, mm_dtype)
    AF = mybir.ActivationFunctionType

    nc = bacc.Bacc("TRN2", target_bir_lowering=False, debug=False)

    xq = nc.dram_tensor("xq", [D, S], mmdt, kind="ExternalInput").ap()
    xk = nc.dram_tensor("xk", [D, S], mmdt, kind="ExternalInput").ap()
    xv = nc.dram_tensor("xv", [D, S], mmdt, kind="ExternalInput").ap()
    wq = nc.dram_tensor("wq", [128, 2048], mmdt, kind="ExternalInput").ap()
    wk = nc.dram_tensor("wk", [128, 2048], mmdt, kind="ExternalInput").ap()
    wv = nc.dram_tensor("wv", [128, 2048], mmdt, kind="ExternalInput").ap()
    wo = nc.dram_tensor("wo", [128, 2048], mmdt, kind="ExternalInput").ap()
    bias6 = nc.dram_tensor("bias6", [128, 10], f32, kind="ExternalInput").ap()
    c_ident = nc.dram_tensor("c_ident", [128, 64], mmdt, kind="ExternalInput").ap()
    pout = nc.dram_tensor("pout", [D, S], mmdt, kind="ExternalOutput").ap()

    with tile.TileContext(nc) as tc:
        from contextlib import ExitStack

        with ExitStack() as stk:
            const = stk.enter_context(tc.tile_pool(name="const", bufs=1))
            wpool = stk.enter_context(tc.tile_pool(name="w", bufs=1))
            big = stk.enter_context(tc.tile_pool(name="big", bufs=1))
            xpool = stk.enter_context(tc.tile_pool(name="xt", bufs=12))
            ptp = stk.enter_context(tc.tile_pool(name="pt", bufs=4))
            rsp = stk.enter_context(tc.tile_pool(name="rs", bufs=2))
            stp = stk.enter_context(tc.tile_pool(name="st", bufs=4))

            # ---- constants ----
            ident = const.tile([128, 64], mmdt)
            nc.sync.dma_start(out=ident, in_=c_ident)
            bias_sb = const.tile([128, 10], f32)
            nc.sync.dma_start(out=bias_sb, in_=bias6)
            # preload the exp table set while projections run
            warm = const.tile([128, 1], f32)
            nc.scalar.activation(warm, bias_sb[:, 0:1], AF.Exp, scale=0.0)

            def dma_split(dst, src, nq=4):
                """Split a [128, N] HBM->SBUF load across nq DMA queues
                (per-queue BW is ~23 GB/s; a 512KB tile on one queue = 23us)."""
                step = 128 // nq
                for q in range(nq):
                    nc.sync.dma_start(
                        out=dst[q * step : (q + 1) * step, :],
                        in_=src[q * step : (q + 1) * step, :],
                    )

            # ---- weights (wo deferred until the out-projection) ----
            wq_sb = wpool.tile([128, 2048], mmdt)
            wk_sb = wpool.tile([128, 2048], mmdt)
            wv_sb = wpool.tile([128, 2048], mmdt)
            wo_sb = wpool.tile([128, 2048], mmdt)
            dma_split(wk_sb, wk)
            dma_split(wq_sb, wq)
            dma_split(wv_sb, wv)

            # ---- persistent activations ----
            qT2 = big.tile([128, 4096], mmdt)  # [dh within ob, ob*2048 + tok]
            vT2 = big.tile([128, 4096], mmdt)
            oT2 = big.tile([128, 4096], mmdt)
            # per-head zero-padded K: kpads[h] holds head h's k rows in
            # partitions h*64:(h+1)*64, zeros elsewhere
            kpA = big.tile([128, 4096], mmdt)
            kpB = big.tile([128, 4096], mmdt)
            kpads = [kpA, kpB]
            # v_aug blocks [j, 128]: cols 0:64 = V block, cols 64:128 = 1.0
            v_sb = big.tile([128, 4 * 16 * 128], mmdt)
            v_r = v_sb.rearrange("p (t c) -> p t c", c=128)
            nc.vector.memset(v_r[:, :, 64:128], 1.0)

            def emit_proj(x_dram, w_sb, pnm, pp, writeback):
                """acc[ob] = W[:, ob].T @ x for both ob blocks; `writeback(ob,
                n, acc)` copies psum->SBUF.  Per-ob acc quads rotate through
                the shared 8-slot pool so projections pipeline stall-free."""
                x_ts = []
                for kk in range(8):
                    x_t = xpool.tile([128, 2048], mmdt, tag="xt", name=f"x{pnm}{kk}")
                    dma_split(x_t, x_dram[kk * 128 : (kk + 1) * 128, :])
                    x_ts.append(x_t)
                for ob in range(2):
                    acc = [
                        pp.tile([128, 512], f32, tag="pp", name=f"acc{pnm}{ob}_{n}")
                        for n in range(4)
                    ]
                    for kk in range(8):
                        for n in range(4):
                            nc.tensor.matmul(
                                acc[n],
                                lhsT=w_sb[:, (kk * 2 + ob) * 128 : (kk * 2 + ob + 1) * 128],
                                rhs=x_ts[kk][:, n * 512 : (n + 1) * 512],
                                start=(kk == 0),
                                stop=(kk == 7),
                            )
                    for n in range(4):
                        writeback(ob, n, acc[n])

            def wb_simple(dst, bias_col0):
                def wb(ob, n, acc):
                    dstv = dst[:, ob * 2048 + n * 512 : ob * 2048 + (n + 1) * 512]
                    bv = bias_sb[:, bias_col0 + ob : bias_col0 + ob + 1]
                    if n < 2:
                        nc.vector.tensor_scalar_add(dstv, acc, bv)
                    else:
                        nc.scalar.activation(dstv, acc, AF.Identity, bias=bv)
                return wb

            ALU = mybir.AluOpType

            def wb_kpad(ob, n, acc):
                """Full-row masked writes build the per-head zero-padded K
                tiles with no separate memset: kpX = acc*maskX + bk*maskX."""
                cs = slice(ob * 2048 + n * 512, ob * 2048 + (n + 1) * 512)
                nc.vector.tensor_scalar(
                    kpA[:, cs], acc,
                    scalar1=bias_sb[:, 8:9], scalar2=bias_sb[:, 2 + ob : 3 + ob],
                    op0=ALU.mult, op1=ALU.add,
                )
                nc.scalar.activation(
                    kpB[:, cs], acc, AF.Identity,
                    bias=bias_sb[:, 4 + ob : 5 + ob], scale=bias_sb[:, 9:10],
                )

            def emit_transp():
                """vT2 -> v_sb [j, d] blocks (cols 64:128 stay 1.0)."""
                with tc.tile_pool(name="tp", bufs=3, space="PSUM") as tpp:
                    for ob in range(2):
                        for h in range(2):
                            bh = ob * 2 + h
                            for g in range(4):
                                tp = tpp.tile(
                                    [128, 4 * 64], mmdt, tag="tp", name=f"tp{bh}_{g}"
                                )
                                for u in range(4):
                                    jb = g * 4 + u
                                    nc.tensor.transpose(
                                        tp[:, u * 64 : (u + 1) * 64],
                                        vT2[
                                            h * 64 : (h + 1) * 64,
                                            ob * 2048 + jb * 128 : ob * 2048 + (jb + 1) * 128,
                                        ],
                                        ident[h * 64 : (h + 1) * 64, :],
                                    )
                                tp_r = tp.rearrange("p (t c) -> p t c", c=64)
                                nc.scalar.copy(
                                    v_r[:, bh * 16 + g * 4 : bh * 16 + g * 4 + 4, 0:64],
                                    tp_r,
                                )

            # =========== emission schedule ===========
            with tc.tile_pool(name="pp", bufs=8, space="PSUM") as pp:
                emit_proj(xk, wk_sb, "k", pp, wb_kpad)
                emit_proj(xq, wq_sb, "q", pp, wb_simple(qT2, 0))
                emit_proj(xv, wv_sb, "v", pp, wb_simple(vT2, 6))
            emit_transp()

            # ---- attention: pipelined over (ob, half, jt-pair) ----
            def emit_fin_stage(o_ps, tag):
                """Rows 64:128 all hold the row-sum (1.0-padded v columns):
                stage to SBUF (custom-DVE ops ignore input base partition),
                then fast-reciprocal."""
                Rss = []
                for h in range(2):
                    rsum = rsp.tile([64, 1024], f32, tag="rsm", name=f"rm{tag}_{h}")
                    nc.vector.tensor_copy(rsum, o_ps[h][64:128, :])
                    Rs = rsp.tile([64, 1024], f32, tag="rs", name=f"Rs{tag}_{h}")
                    nc.vector.reciprocal_approx_fast(Rs, rsum)
                    Rss.append(Rs)
                return Rss

            def emit_fin_mul(o_ps, Rss, i0, tag):
                for h in range(2):
                    nc.vector.tensor_mul(
                        oT2[h * 64 : (h + 1) * 64, i0 : i0 + 1024],
                        o_ps[h][0:64, :],
                        Rss[h],
                    )

            with (
                tc.tile_pool(name="scp", bufs=2, space="PSUM") as scp,
                tc.tile_pool(name="opp", bufs=2, space="PSUM") as opp,
            ):
                pending = None
                for ob in range(2):
                    for half in range(2):
                        i0 = ob * 2048 + half * 1024
                        o_ps = [
                            opp.tile([128, 1024], f32, tag="ops", name=f"o{ob}_{half}_{h}")
                            for h in range(2)
                        ]
                        pt_hist = {}

                        def emit_pv(p):
                            for h in range(2):
                                bh = ob * 2 + h
                                for b in range(2):
                                    jt = 2 * p + b
                                    for c in range(2):
                                        nc.tensor.matmul(
                                            o_ps[h][:, c * 512 : (c + 1) * 512],
                                            lhsT=v_r[:, bh * 16 + jt, :],
                                            rhs=pt_hist[p][h][:, b, c * 512 : (c + 1) * 512],
                                            start=(jt == 0),
                                            stop=(jt == 15),
                                        )
                            del pt_hist[p]

                        for p in range(8):
                            pt_hist[p] = [
                                ptp.tile(
                                    [128, 2, 1024], mmdt, tag="pt",
                                    name=f"p{ob}_{half}_{p}_{h}",
                                )
                                for h in range(2)
                            ]
                            for b in range(2):
                                jt = 2 * p + b
                                # PV of pair p-1 rides one sub-step behind so
                                # its exp inputs are long done: no PE stall
                                if b == 1 and p >= 1:
                                    emit_pv(p - 1)
                                sc = [
                                    scp.tile(
                                        [128, 1024], f32, tag="sc",
                                        name=f"s{ob}_{half}_{jt}_{h}",
                                    )
                                    for h in range(2)
                                ]
                                for c in range(2):
                                    for h in range(2):
                                        nc.tensor.matmul(
                                            sc[h][:, c * 512 : (c + 1) * 512],
                                            lhsT=kpads[h][
                                                :,
                                                ob * 2048 + jt * 128 : ob * 2048 + (jt + 1) * 128,
                                            ],
                                            rhs=qT2[:, i0 + c * 512 : i0 + (c + 1) * 512],
                                            start=True,
                                            stop=True,
                                        )
                                for h in range(2):
                                    nc.scalar.activation(
                                        pt_hist[p][h][:, b, :], sc[h], AF.Exp, scale=0.125
                                    )
                            # deferred normalize of the previous (ob, half):
                            # its reciprocal is already done
                            if p == 0 and pending is not None:
                                emit_fin_mul(*pending)
                                pending = None
                        emit_pv(7)
                        Rss = emit_fin_stage(o_ps, f"{ob}_{half}")
                        pending = (o_ps, Rss, i0, f"{ob}_{half}")
                emit_fin_mul(*pending)

            # ---- out-projection: partial = Wo_slice.T @ oT (K=256) ----
            dma_split(wo_sb, wo)
            with tc.tile_pool(name="opj", bufs=8, space="PSUM") as pj:
                for dtb in range(8):
                    ops = [
                        pj.tile([128, 512], f32, tag="pj", name=f"pj{dtb}_{c}")
                        for c in range(4)
                    ]
                    for ob in range(2):
                        for c in range(4):
                            nc.tensor.matmul(
                                ops[c],
                                lhsT=wo_sb[:, (ob * 8 + dtb) * 128 : (ob * 8 + dtb + 1) * 128],
                                rhs=oT2[:, ob * 2048 + c * 512 : ob * 2048 + (c + 1) * 512],
                                start=(ob == 0),
                                stop=(ob == 1),
                            )
                    for c in range(4):
                        st = stp.tile([128, 512], mmdt, tag="st", name=f"st{dtb}_{c}")
                        eng = nc.vector.tensor_copy if c % 2 else nc.scalar.copy
                        eng(st, ops[c])
                        nc.sync.dma_start(
                            out=pout[
                                dtb * 128 : (dtb + 1) * 128, c * 512 : (c + 1) * 512
                            ],
                            in_=st,
                        )

    nc.compile()
    return nc


MM_DTYPE = "float16"


def _get_nc():
    key = ("nc", MM_DTYPE)
    if key not in _CACHE:
        _CACHE[key] = _build_nc(MM_DTYPE)
    return _CACHE[key]


def _ensure_ntff_hook():
    """Register the NTFF profile hook module if the image lacks it."""
    import sys
    import types

    if "antenv.axon_hooks" in sys.modules:
        return
    try:
        from trn_agent_boot.trn_boot import _ntff_profile_via_ctypes
    except Exception:
        return
    hook = None
    try:
        hook = _ntff_profile_via_ctypes("/opt/axon/libaxon_pjrt.so")
    except Exception:
        hook = None
    mod = types.ModuleType("antenv.axon_hooks")
    mod._hook = hook
    mod.get_axon_ntff_profile_hook = lambda: mod._hook
    mod.set_axon_ntff_profile_hook = lambda h: setattr(mod, "_hook", h)
    sys.modules["antenv.axon_hooks"] = mod


def _make_in_maps(inputs, ext_dt):
    query = np.asarray(inputs["query"], np.float32)
    key = np.asarray(inputs["key"], np.float32)
    value = np.asarray(inputs["value"], np.float32)
    Wq = np.asarray(inputs["Wq"], np.float32)
    Wk = np.asarray(inputs["Wk"], np.float32)
    Wv = np.asarray(inputs["Wv"], np.float32)
    Wo = np.asarray(inputs["Wo"], np.float32)
    bq = np.asarray(inputs["bq"], np.float32)
    bk = np.asarray(inputs["bk"], np.float32)
    bv = np.asarray(inputs["bv"], np.float32)

    # per-batch transposed inputs [D, S]
    xT = {}
    for b in range(B):
        xT[("q", b)] = np.ascontiguousarray(query[b].T.astype(ext_dt))
        xT[("k", b)] = np.ascontiguousarray(key[b].T.astype(ext_dt))
        xT[("v", b)] = np.ascontiguousarray(value[b].T.astype(ext_dt))

    ident_np = np.zeros((128, 64), np.float32)
    ident_np[np.arange(64), np.arange(64)] = 1.0
    ident_np[64 + np.arange(64), np.arange(64)] = 1.0
    consts = {
        "c_ident": np.ascontiguousarray(ident_np.astype(ext_dt)),
    }
    maskA = (np.arange(128) < 64).astype(np.float32)
    maskB = 1.0 - maskA

    def pack_w(Wc):  # [1024, 256] -> [128, 2048] as (kk, ob) tiles
        return np.ascontiguousarray(
            Wc.reshape(8, 128, 2, 128).transpose(1, 0, 2, 3).reshape(128, 2048).astype(ext_dt)
        )

    def pack_wo(Wc):  # [256, 1024] -> [128, 2048] as (ob, dt) tiles
        return np.ascontiguousarray(
            Wc.reshape(2, 128, 8, 128).transpose(1, 0, 2, 3).reshape(128, 2048).astype(ext_dt)
        )

    in_maps = []
    for c in range(N_CORES):
        b, hs = divmod(c, 4)
        sl = slice(hs * 256, (hs + 1) * 256)
        bias6 = np.zeros((128, 10), np.float32)
        bias6[:, 0] = bq[sl][0:128]
        bias6[:, 1] = bq[sl][128:256]
        bias6[:, 2] = bk[sl][0:128] * maskA
        bias6[:, 3] = bk[sl][128:256] * maskA
        bias6[:, 4] = bk[sl][0:128] * maskB
        bias6[:, 5] = bk[sl][128:256] * maskB
        bias6[:, 6] = bv[sl][0:128]
        bias6[:, 7] = bv[sl][128:256]
        bias6[:, 8] = maskA
        bias6[:, 9] = maskB
        in_maps.append(
            {
                **consts,
                "xq": xT[("q", b)],
                "xk": xT[("k", b)],
                "xv": xT[("v", b)],
                "wq": pack_w(Wq[:, sl]),
                "wk": pack_w(Wk[:, sl]),
                "wv": pack_w(Wv[:, sl]),
                "wo": pack_wo(Wo[sl, :]),
                "bias6": np.ascontiguousarray(bias6),
            }
        )
    return in_maps


def _gather(results, bo):
    outT = np.zeros((B, D, S), np.float64)
    for c in range(N_CORES):
        outT[c // 4] += np.asarray(results[c]["pout"], np.float64)
    out = outT.transpose(0, 2, 1) + bo.astype(np.float64)
    return out.astype(np.float32)


def _run(inputs, trace=False):
    from concourse import bass_utils

    if trace:
        _ensure_ntff_hook()

    nc = _get_nc()
    if MM_DTYPE == "bfloat16":
        import ml_dtypes

        ext_dt = ml_dtypes.bfloat16
    elif MM_DTYPE == "float16":
        ext_dt = np.float16
    else:
        ext_dt = np.float32

    in_maps = _make_in_maps(inputs, ext_dt)
    res = bass_utils.run_bass_kernel_spmd(
        nc, in_maps, core_ids=list(range(N_CORES)), trace=trace
    )
    bo = np.asarray(inputs["bo"], np.float32)
    out = _gather(res.results, bo)
    return out.reshape(B, S, D), res


def kernel(**inputs):
    out, _ = _run(inputs, trace=False)
    return out


# revision 29
# speedup vs baseline: 1.0517x; 1.0517x over previous
"""Multi-head attention (B=2, S=2048, D=1024, H=16, Dk=64) on 8 TRN2 cores.

Sharding: batch-split x head-TP.  Core c handles batch c//4 and heads
hs*4..hs*4+3 where hs = c%4 (256 projection dims = 2 "ob" blocks of 128).

The PE clock-gate (HAM) only unthrottles for full-array matmuls, so every
attention matmul is padded to 128x128:
  - scores: per-head K tiles kpA/kpB hold the head's 64 k-dims zero-padded
    to 128 partitions (zeros annihilate the other head's q rows), so
    scoresT = kpad.T @ qT runs K=128 full-array.
  - PV: v_aug columns padded with 1.0 to M=128; PSUM rows 0:64 = o,
    rows 64:128 all = softmax row-sum (the 1-columns), which feeds
    reciprocal_approx_fast directly -- no broadcast matmul needed.
Each core:
  1. projects k/q/v = (W_slice.T @ x.T) for its 4 heads
  2. transposes vT into per-(ob,h) [j, d] blocks (cols 64:128 = 1.0)
  3. pipelined attention per (ob, half): scoresT -> exp (FD=1024 ACT)
     -> PV accumulate [128, 1024] PSUM -> 1/rowsum -> normalize into oT2
  4. partialT = Wo_slice.T @ oT  (K=256 accumulated over both obs)
Host sums 4 partials per batch, adds bo, transposes back.
All matmuls fp16 operands with fp32 PSUM accumulation.
"""

import numpy as np

D = 1024
S = 2048  # tokens per batch (= per core)
B = 2
N_CORES = 8

_CACHE = {}


def _build_nc(mm_dtype="float16"):
    import concourse.bacc as bacc
    import concourse.mybir as mybir
    import concourse.tile as tile

    dt = mybir.dt
    f32 = dt.float32
    mmdt = getattr(dt, mm_dtype)
    AF = mybir.ActivationFunctionType

    nc = bacc.Bacc("TRN2", target_bir_lowering=False, debug=False)

    xq = nc.dram_tensor("xq", [D, S], mmdt, kind="ExternalInput").ap()
    xk = nc.dram_tensor("xk", [D, S], mmdt, kind="ExternalInput").ap()
    xv = nc.dram_tensor("xv", [D, S], mmdt, kind="ExternalInput").ap()
    wq = nc.dram_tensor("wq", [128, 2048], mmdt, kind="ExternalInput").ap()
    wk = nc.dram_tensor("wk", [128, 2048], mmdt, kind="ExternalInput").ap()
    wv = nc.dram_tensor("wv", [128, 2048], mmdt, kind="ExternalInput").ap()
    wo = nc.dram_tensor("wo", [128, 2048], mmdt, kind="ExternalInput").ap()
    bias6 = nc.dram_tensor("bias6", [128, 10], f32, kind="ExternalInput").ap()
    c_ident = nc.dram_tensor("c_ident", [128, 64], mmdt, kind="ExternalInput").ap()
    pout = nc.dram_tensor("pout", [D, S], mmdt, kind="ExternalOutput").ap()

    with tile.TileContext(nc) as tc:
        from contextlib import ExitStack

        with ExitStack() as stk:
            const = stk.enter_context(tc.tile_pool(name="const", bufs=1))
            wpool = stk.enter_context(tc.tile_pool(name="w", bufs=1))
            big = stk.enter_context(tc.tile_pool(name="big", bufs=1))
            xpool = stk.enter_context(tc.tile_pool(name="xt", bufs=12))
            ptp = stk.enter_context(tc.tile_pool(name="pt", bufs=4))
            rsp = stk.enter_context(tc.tile_pool(name="rs", bufs=2))
            stp = stk.enter_context(tc.tile_pool(name="st", bufs=4))

            # ---- constants ----
            ident = const.tile([128, 64], mmdt)
            nc.sync.dma_start(out=ident, in_=c_ident)
            bias_sb = const.tile([128, 10], f32)
            nc.sync.dma_start(out=bias_sb, in_=bias6)
            # preload the exp table set while projections run
            warm = const.tile([128, 1], f32)
            nc.scalar.activation(warm, bias_sb[:, 0:1], AF.Exp, scale=0.0)

            def dma_split(dst, src, nq=4, engs=None):
                """Split a [128, N] HBM->SBUF load across nq DMA queues
                (per-queue BW is ~23 GB/s; a 512KB tile on one queue = 23us).
                `engs` spreads the ~600ns descriptor-gen cost across issue
                queues (sync alone serializes at ~600ns per dma_start)."""
                step = 128 // nq
                if engs is None:
                    engs = [nc.sync]
                for q in range(nq):
                    engs[q % len(engs)].dma_start(
                        out=dst[q * step : (q + 1) * step, :],
                        in_=src[q * step : (q + 1) * step, :],
                    )

            # ---- weights (wo deferred until the out-projection) ----
            wq_sb = wpool.tile([128, 2048], mmdt)
            wk_sb = wpool.tile([128, 2048], mmdt)
            wv_sb = wpool.tile([128, 2048], mmdt)
            wo_sb = wpool.tile([128, 2048], mmdt)
            dma_split(wk_sb, wk, engs=[nc.gpsimd, nc.sync])

            # ---- persistent activations ----
            qT2 = big.tile([128, 4096], mmdt)  # [dh within ob, ob*2048 + tok]
            vT2 = big.tile([128, 4096], mmdt)
            oT2 = big.tile([128, 4096], mmdt)
            # per-head zero-padded K: kpads[h] holds head h's k rows in
            # partitions h*64:(h+1)*64, zeros elsewhere
            kpA = big.tile([128, 4096], mmdt)
            kpB = big.tile([128, 4096], mmdt)
            kpads = [kpA, kpB]
            # v_aug blocks [j, 128]: cols 0:64 = V block, cols 64:128 = 1.0
            v_sb = big.tile([128, 4 * 16 * 128], mmdt)
            v_r = v_sb.rearrange("p (t c) -> p t c", c=128)
            nc.vector.memset(v_r[:, :, 64:128], 1.0)

            def emit_proj(x_dram, w_sb, pnm, pp, writeback):
                """acc[ob] = W[:, ob].T @ x for both ob blocks; `writeback(ob,
                n, acc)` copies psum->SBUF.  Per-ob acc quads rotate through
                the shared 8-slot pool so projections pipeline stall-free."""
                x_ts = []
                for kk in range(8):
                    x_t = xpool.tile([128, 2048], mmdt, tag="xt", name=f"x{pnm}{kk}")
                    dma_split(
                        x_t, x_dram[kk * 128 : (kk + 1) * 128, :],
                        engs=[nc.sync, nc.gpsimd],
                    )
                    x_ts.append(x_t)
                for ob in range(2):
                    acc = [
                        pp.tile([128, 512], f32, tag="pp", name=f"acc{pnm}{ob}_{n}")
                        for n in range(4)
                    ]
                    for kk in range(8):
                        for n in range(4):
                            nc.tensor.matmul(
                                acc[n],
                                lhsT=w_sb[:, (kk * 2 + ob) * 128 : (kk * 2 + ob + 1) * 128],
                                rhs=x_ts[kk][:, n * 512 : (n + 1) * 512],
                                start=(kk == 0),
                                stop=(kk == 7),
                            )
                    for n in range(4):
                        writeback(ob, n, acc[n])

            def wb_simple(dst, bias_col0):
                def wb(ob, n, acc):
                    dstv = dst[:, ob * 2048 + n * 512 : ob * 2048 + (n + 1) * 512]
                    bv = bias_sb[:, bias_col0 + ob : bias_col0 + ob + 1]
                    if n < 2:
                        nc.vector.tensor_scalar_add(dstv, acc, bv)
                    else:
                        nc.scalar.activation(dstv, acc, AF.Identity, bias=bv)
                return wb

            ALU = mybir.AluOpType

            def wb_kpad(ob, n, acc):
                """Full-row masked writes build the per-head zero-padded K
                tiles with no separate memset: kpX = acc*maskX + bk*maskX."""
                cs = slice(ob * 2048 + n * 512, ob * 2048 + (n + 1) * 512)
                nc.vector.tensor_scalar(
                    kpA[:, cs], acc,
                    scalar1=bias_sb[:, 8:9], scalar2=bias_sb[:, 2 + ob : 3 + ob],
                    op0=ALU.mult, op1=ALU.add,
                )
                nc.scalar.activation(
                    kpB[:, cs], acc, AF.Identity,
                    bias=bias_sb[:, 4 + ob : 5 + ob], scale=bias_sb[:, 9:10],
                )

            def emit_transp():
                """vT2 -> v_sb [j, d] blocks (cols 64:128 stay 1.0)."""
                with tc.tile_pool(name="tp", bufs=3, space="PSUM") as tpp:
                    for ob in range(2):
                        for h in range(2):
                            bh = ob * 2 + h
                            for g in range(4):
                                tp = tpp.tile(
                                    [128, 4 * 64], mmdt, tag="tp", name=f"tp{bh}_{g}"
                                )
                                for u in range(4):
                                    jb = g * 4 + u
                                    nc.tensor.transpose(
                                        tp[:, u * 64 : (u + 1) * 64],
                                        vT2[
                                            h * 64 : (h + 1) * 64,
                                            ob * 2048 + jb * 128 : ob * 2048 + (jb + 1) * 128,
                                        ],
                                        ident[h * 64 : (h + 1) * 64, :],
                                    )
                                tp_r = tp.rearrange("p (t c) -> p t c", c=64)
                                nc.scalar.copy(
                                    v_r[:, bh * 16 + g * 4 : bh * 16 + g * 4 + 4, 0:64],
                                    tp_r,
                                )

            # =========== emission schedule ===========
            with tc.tile_pool(name="pp", bufs=8, space="PSUM") as pp:
                emit_proj(xk, wk_sb, "k", pp, wb_kpad)
                dma_split(wq_sb, wq, engs=[nc.gpsimd, nc.sync])
                emit_proj(xq, wq_sb, "q", pp, wb_simple(qT2, 0))
                dma_split(wv_sb, wv, engs=[nc.gpsimd, nc.sync])
                emit_proj(xv, wv_sb, "v", pp, wb_simple(vT2, 6))
            emit_transp()
            dma_split(wo_sb, wo, engs=[nc.gpsimd, nc.sync])

            # ---- attention: pipelined over (ob, half, jt-pair) ----
            def emit_fin_stage(o_ps, tag, use_act=False):
                """Rows 64:128 all hold the row-sum (1.0-padded v columns):
                stage to SBUF (custom-DVE ops ignore input base partition),
                then fast-reciprocal.  The last generation stages on ACT
                (idle post-attention) to shorten the serial DVE tail."""
                Rss = []
                for h in range(2):
                    rsum = rsp.tile([64, 1024], f32, tag="rsm", name=f"rm{tag}_{h}")
                    if use_act:
                        nc.scalar.copy(rsum, o_ps[h][64:128, :])
                    else:
                        nc.vector.tensor_copy(rsum, o_ps[h][64:128, :])
                    Rs = rsp.tile([64, 1024], f32, tag="rs", name=f"Rs{tag}_{h}")
                    nc.vector.reciprocal_approx_fast(Rs, rsum)
                    Rss.append(Rs)
                return Rss

            def emit_fin_mul(o_ps, Rss, i0, tag):
                for h in range(2):
                    nc.vector.tensor_mul(
                        oT2[h * 64 : (h + 1) * 64, i0 : i0 + 1024],
                        o_ps[h][0:64, :],
                        Rss[h],
                    )

            with (
                tc.tile_pool(name="scp", bufs=2, space="PSUM") as scp,
                tc.tile_pool(name="opp", bufs=2, space="PSUM") as opp,
            ):
                pending = None
                for ob in range(2):
                    for half in range(2):
                        i0 = ob * 2048 + half * 1024
                        o_ps = [
                            opp.tile([128, 1024], f32, tag="ops", name=f"o{ob}_{half}_{h}")
                            for h in range(2)
                        ]
                        pt_hist = {}

                        def emit_pv(p):
                            for h in range(2):
                                bh = ob * 2 + h
                                for b in range(2):
                                    jt = 2 * p + b
                                    for c in range(2):
                                        nc.tensor.matmul(
                                            o_ps[h][:, c * 512 : (c + 1) * 512],
                                            lhsT=v_r[:, bh * 16 + jt, :],
                                            rhs=pt_hist[p][h][:, b, c * 512 : (c + 1) * 512],
                                            start=(jt == 0),
                                            stop=(jt == 15),
                                        )
                            del pt_hist[p]

                        for p in range(8):
                            pt_hist[p] = [
                                ptp.tile(
                                    [128, 2, 1024], mmdt, tag="pt",
                                    name=f"p{ob}_{half}_{p}_{h}",
                                )
                                for h in range(2)
                            ]
                            for b in range(2):
                                jt = 2 * p + b
                                # PV of pair p-1 rides one sub-step behind so
                                # its exp inputs are long done: no PE stall
                                if b == 1 and p >= 1:
                                    emit_pv(p - 1)
                                sc = [
                                    scp.tile(
                                        [128, 1024], f32, tag="sc",
                                        name=f"s{ob}_{half}_{jt}_{h}",
                                    )
                                    for h in range(2)
                                ]
                                for c in range(2):
                                    for h in range(2):
                                        nc.tensor.matmul(
                                            sc[h][:, c * 512 : (c + 1) * 512],
                                            lhsT=kpads[h][
                                                :,
                                                ob * 2048 + jt * 128 : ob * 2048 + (jt + 1) * 128,
                                            ],
                                            rhs=qT2[:, i0 + c * 512 : i0 + (c + 1) * 512],
                                            start=True,
                                            stop=True,
                                        )
                                for h in range(2):
                                    nc.scalar.activation(
                                        pt_hist[p][h][:, b, :], sc[h], AF.Exp, scale=0.125
                                    )
                            # deferred normalize of the previous (ob, half):
                            # its reciprocal is already done
                            if p == 0 and pending is not None:
                                emit_fin_mul(*pending)
                                pending = None
                        emit_pv(7)
                        Rss = emit_fin_stage(
                            o_ps, f"{ob}_{half}", use_act=(ob == 1 and half == 1)
                        )
                        pending = (o_ps, Rss, i0, f"{ob}_{half}")
                emit_fin_mul(*pending)

                # ---- out-projection inside the attention pools: accs reuse
                # the sc/ops PSUM slots, so no pool-alloc barrier serializes
                # the start behind the final finalize ----
                for dtb in range(8):
                    ops = [
                        (scp if c < 2 else opp).tile(
                            [128, 512], f32,
                            tag=("sc" if c < 2 else "ops"),
                            name=f"pj{dtb}_{c}",
                        )
                        for c in range(4)
                    ]
                    for ob in range(2):
                        for c in range(4):
                            nc.tensor.matmul(
                                ops[c],
                                lhsT=wo_sb[:, (ob * 8 + dtb) * 128 : (ob * 8 + dtb + 1) * 128],
                                rhs=oT2[:, ob * 2048 + c * 512 : ob * 2048 + (c + 1) * 512],
                                start=(ob == 0),
                                stop=(ob == 1),
                            )
                    for c in range(4):
                        st = stp.tile([128, 512], mmdt, tag="st", name=f"st{dtb}_{c}")
                        eng = nc.vector.tensor_copy if c % 2 else nc.scalar.copy
                        eng(st, ops[c])
                        nc.sync.dma_start(
                            out=pout[
                                dtb * 128 : (dtb + 1) * 128, c * 512 : (c + 1) * 512
                            ],
                            in_=st,
                        )


    nc.compile()
    return nc


MM_DTYPE = "float16"


def _get_nc():
    key = ("nc", MM_DTYPE)
    if key not in _CACHE:
        _CACHE[key] = _build_nc(MM_DTYPE)
    return _CACHE[key]


def _ensure_ntff_hook():
    """Register the NTFF profile hook module if the image lacks it."""
    import sys
    import types

    if "antenv.axon_hooks" in sys.modules:
        return
    try:
        from trn_agent_boot.trn_boot import _ntff_profile_via_ctypes
    except Exception:
        return
    hook = None
    try:
        hook = _ntff_profile_via_ctypes("/opt/axon/libaxon_pjrt.so")
    except Exception:
        hook = None
    mod = types.ModuleType("antenv.axon_hooks")
    mod._hook = hook
    mod.get_axon_ntff_profile_hook = lambda: mod._hook
    mod.set_axon_ntff_profile_hook = lambda h: setattr(mod, "_hook", h)
    sys.modules["antenv.axon_hooks"] = mod


def _make_in_maps(inputs, ext_dt):
    query = np.asarray(inputs["query"], np.float32)
    key = np.asarray(inputs["key"], np.float32)
    value = np.asarray(inputs["value"], np.float32)
    Wq = np.asarray(inputs["Wq"], np.float32)
    Wk = np.asarray(inputs["Wk"], np.float32)
    Wv = np.asarray(inputs["Wv"], np.float32)
    Wo = np.asarray(inputs["Wo"], np.float32)
    bq = np.asarray(inputs["bq"], np.float32)
    bk = np.asarray(inputs["bk"], np.float32)
    bv = np.asarray(inputs["bv"], np.float32)

    # per-batch transposed inputs [D, S]
    xT = {}
    for b in range(B):
        xT[("q", b)] = np.ascontiguousarray(query[b].T.astype(ext_dt))
        xT[("k", b)] = np.ascontiguousarray(key[b].T.astype(ext_dt))
        xT[("v", b)] = np.ascontiguousarray(value[b].T.astype(ext_dt))

    ident_np = np.zeros((128, 64), np.float32)
    ident_np[np.arange(64), np.arange(64)] = 1.0
    ident_np[64 + np.arange(64), np.arange(64)] = 1.0
    consts = {
        "c_ident": np.ascontiguousarray(ident_np.astype(ext_dt)),
    }
    maskA = (np.arange(128) < 64).astype(np.float32)
    maskB = 1.0 - maskA

    def pack_w(Wc):  # [1024, 256] -> [128, 2048] as (kk, ob) tiles
        return np.ascontiguousarray(
            Wc.reshape(8, 128, 2, 128).transpose(1, 0, 2, 3).reshape(128, 2048).astype(ext_dt)
        )

    def pack_wo(Wc):  # [256, 1024] -> [128, 2048] as (ob, dt) tiles
        return np.ascontiguousarray(
            Wc.reshape(2, 128, 8, 128).transpose(1, 0, 2, 3).reshape(128, 2048).astype(ext_dt)
        )

    in_maps = []
    for c in range(N_CORES):
        b, hs = divmod(c, 4)
        sl = slice(hs * 256, (hs + 1) * 256)
        bias6 = np.zeros((128, 10), np.float32)
        bias6[:, 0] = bq[sl][0:128]
        bias6[:, 1] = bq[sl][128:256]
        bias6[:, 2] = bk[sl][0:128] * maskA
        bias6[:, 3] = bk[sl][128:256] * maskA
        bias6[:, 4] = bk[sl][0:128] * maskB
        bias6[:, 5] = bk[sl][128:256] * maskB
        bias6[:, 6] = bv[sl][0:128]
        bias6[:, 7] = bv[sl][128:256]
        bias6[:, 8] = maskA
        bias6[:, 9] = maskB
        in_maps.append(
            {
                **consts,
                "xq": xT[("q", b)],
                "xk": xT[("k", b)],
                "xv": xT[("v", b)],
                "wq": pack_w(Wq[:, sl]),
                "wk": pack_w(Wk[:, sl]),
                "wv": pack_w(Wv[:, sl]),
                "wo": pack_wo(Wo[sl, :]),
                "bias6": np.ascontiguousarray(bias6),
            }
        )
    return in_maps


def _gather(results, bo):
    outT = np.zeros((B, D, S), np.float64)
    for c in range(N_CORES):
        outT[c // 4] += np.asarray(results[c]["pout"], np.float64)
    out = outT.transpose(0, 2, 1) + bo.astype(np.float64)
    return out.astype(np.float32)


def _run(inputs, trace=False):
    from concourse import bass_utils

    if trace:
        _ensure_ntff_hook()

    nc = _get_nc()
    if MM_DTYPE == "bfloat16":
        import ml_dtypes

        ext_dt = ml_dtypes.bfloat16
    elif MM_DTYPE == "float16":
        ext_dt = np.float16
    else:
        ext_dt = np.float32

    in_maps = _make_in_maps(inputs, ext_dt)
    res = bass_utils.run_bass_kernel_spmd(
        nc, in_maps, core_ids=list(range(N_CORES)), trace=trace
    )
    bo = np.asarray(inputs["bo"], np.float32)
    out = _gather(res.results, bo)
    return out.reshape(B, S, D), res


def kernel(**inputs):
    out, _ = _run(inputs, trace=False)
    return out
